# revision 1
# baseline (speedup 1.0000x reference)
"""Trainium2 Bass kernel for clustered (sorted-group) multi-head attention.

Full inputs in, full output out. Host does the data-dependent token sort
(argmax over sim + stable argsort), folds the projection weight into v
(vtilde_h = v_h @ W_h, so PV+proj become one PSUM-accumulated matmul pass),
and packs DMA-friendly layouts. The 1024 independent 128-token attention
groups are sharded 128-per-core across 8 NeuronCores.

Device per group and head: S = (scale*q)^T k, rowmax via DVE, exp on Act,
rowsum+recip on DVE, then P-hat^T = P~^T·diag(r) via a single PE matmul
(normalization folded into the transpose), Y = sum_h P-hat_h^T^T @ vtilde_h
accumulated in PSUM, and Y DMA'd from PSUM straight to DRAM in f32.
Bias-add and un-sort on host.
"""

import os
import numpy as np
import ml_dtypes

NUM_HEADS = 4
GS = 128          # tokens per category group
HD = 48           # head dim
CDIM = 192        # channels
B = 2
N = 65536
NCORES = 8
NG = (B * N) // GS            # 1024 total groups
GCORE = NG // NCORES          # 128 groups per core
CHUNK = 16                    # groups per DMA chunk
NCHUNK = GCORE // CHUNK

FW = NUM_HEADS * GS           # 512: per-group free width of q/k staging
VW = NUM_HEADS * CDIM         # 768: per-group free width of vtilde staging

_cache = {}
LAST_RESULT = None

# build-time knobs (see work/ notes)
F_PT_BF16 = False   # diag-matmul must write f32 PSUM (real-matmul constraint)
F_DIAG_POOL = os.environ.get("F_DIAG_POOL", "0") == "1"  # build diag(r) on GpSimd vs DVE


def _build_nc():
    import concourse.bass as bass
    import concourse.mybir as mybir
    from concourse import bacc
    from concourse.tile import TileContext

    dt = mybir.dt
    f32, f16, bf16 = dt.float32, dt.float16, dt.bfloat16

    nc = bacc.Bacc(None, target_bir_lowering=False)
    qt_e = nc.declare_dram_parameter("qt", [HD, NCHUNK, CHUNK * FW], f16, isOutput=False)
    kt_e = nc.declare_dram_parameter("kt", [HD, NCHUNK, CHUNK * FW], f16, isOutput=False)
    vt_e = nc.declare_dram_parameter("vt", [GS, NCHUNK, CHUNK * VW], bf16, isOutput=False)
    id_e = nc.declare_dram_parameter("ident", [GS, GS], bf16, isOutput=False)
    out_e = nc.declare_dram_parameter("out", [GS, GCORE, CDIM], f16, isOutput=True)

    pt_dt = bf16 if F_PT_BF16 else f32

    STAGGER = int(os.environ.get("F_STAGGER", "4"))

    with TileContext(nc) as tc:
        with (
            tc.tile_pool(name="consts", bufs=1) as consts,
            tc.tile_pool(name="qk", bufs=2) as qk_pool,
            tc.tile_pool(name="vp", bufs=2) as v_pool,
            tc.tile_pool(name="op", bufs=2) as o_pool,
            tc.tile_pool(name="pp", bufs=7) as p_pool,
            tc.tile_pool(name="dg", bufs=7) as d_pool,
            tc.tile_pool(name="st", bufs=12) as st_pool,
            tc.tile_pool(name="ps_s", bufs=3, space="PSUM") as ps_s,
            tc.tile_pool(name="ps_t", bufs=2, space="PSUM") as ps_t,
            tc.tile_pool(name="ps_y", bufs=3, space="PSUM") as ps_y,
        ):
            id_t = consts.tile([GS, GS], bf16)
            nc.sync.dma_start(out=id_t, in_=id_e[:, :])

            chunks = {}   # ci -> (q_t, k_t, v_t, out_t)
            state = {}    # g  -> (p4, diag4, v_t, out_t)

            def phase_a(g):
                ci, gi = divmod(g, CHUNK)
                if gi == 0:
                    q_t = qk_pool.tile([HD, CHUNK, FW], f16, tag="q_t")
                    k_t = qk_pool.tile([HD, CHUNK, FW], f16, tag="k_t")
                    v_t = v_pool.tile([GS, CHUNK, NUM_HEADS, CDIM], bf16, tag="v_t")
                    nc.sync.dma_start(
                        out=q_t, in_=qt_e[:, ci].rearrange("p (c f) -> p c f", c=CHUNK))
                    nc.sync.dma_start(
                        out=k_t, in_=kt_e[:, ci].rearrange("p (c f) -> p c f", c=CHUNK))
                    nc.sync.dma_start(
                        out=v_t,
                        in_=vt_e[:, ci].rearrange("p (c h f) -> p c h f", c=CHUNK, h=NUM_HEADS))
                    out_t = o_pool.tile([GS, CHUNK, CDIM], f16)
                    chunks[ci] = (q_t, k_t, v_t, out_t)
                q_t, k_t, v_t, out_t = chunks[ci]
                s4 = ps_s.tile([GS, NUM_HEADS, GS], f32)
                for h in range(NUM_HEADS):
                    nc.tensor.matmul(
                        s4[:, h],
                        lhsT=q_t[:, gi, h * GS : (h + 1) * GS],
                        rhs=k_t[:, gi, h * GS : (h + 1) * GS],
                        start=True, stop=True,
                    )
                negm4 = st_pool.tile([GS, NUM_HEADS], f32, tag="negm")
                nc.vector.tensor_reduce(
                    negm4, s4[:, :, :], axis=mybir.AxisListType.X,
                    op=mybir.AluOpType.max, negate=True,
                )
                p4 = p_pool.tile([GS, NUM_HEADS, GS], bf16, tag="p4")
                for h in range(NUM_HEADS):
                    nc.scalar.activation(
                        p4[:, h], s4[:, h],
                        mybir.ActivationFunctionType.Exp,
                        bias=negm4[:, h : h + 1], scale=1.0,
                    )
                l4 = st_pool.tile([GS, NUM_HEADS], f32, tag="l4")
                nc.vector.tensor_reduce(
                    l4, p4[:, :, :], axis=mybir.AxisListType.X,
                    op=mybir.AluOpType.add,
                )
                r4 = st_pool.tile([GS, NUM_HEADS], f32, tag="r4")
                nc.vector.reciprocal(r4, l4)
                # diag(r_h) on the (otherwise idle) Pool engine
                diag4 = d_pool.tile([GS, NUM_HEADS, GS], bf16, tag="diag4")
                ida, rda = bass.broadcast_tensor_aps(id_t[:, None, :], r4[:, :, None])
                nc.gpsimd.tensor_mul(diag4[:, :, :], ida, rda)
                state[g] = (p4, diag4, v_t, out_t)

            def phase_b(g):
                ci, gi = divmod(g, CHUNK)
                p4, diag4, v_t, out_t = state.pop(g)
                # normalized transpose: pt_h = p4_h^T @ diag(r_h)
                pt4 = ps_t.tile([GS, NUM_HEADS, GS], pt_dt)
                for h in range(NUM_HEADS):
                    nc.tensor.matmul(
                        pt4[:, h], lhsT=p4[:, h], rhs=diag4[:, h],
                        start=True, stop=True,
                    )
                pt_sb = p_pool.tile([GS, NUM_HEADS, GS], bf16, tag="pt_sb")
                nc.vector.tensor_copy(pt_sb[:, :, :], pt4[:, :, :])
                # Y[s, c] += sum_h P-hat^T_h^T @ vtilde_h  (heads in PSUM)
                y = ps_y.tile([GS, CDIM], f32)
                for h in range(NUM_HEADS):
                    nc.tensor.matmul(
                        y,
                        lhsT=pt_sb[:, h],
                        rhs=v_t[:, gi, h],
                        start=(h == 0), stop=(h == NUM_HEADS - 1),
                    )
                nc.scalar.copy(out_t[:, gi], y)
                if gi == CHUNK - 1:
                    nc.sync.dma_start(
                        out=out_e[:, ci * CHUNK : (ci + 1) * CHUNK], in_=out_t
                    )

            for g in range(GCORE + STAGGER):
                if g < GCORE:
                    phase_a(g)
                if g >= STAGGER:
                    phase_b(g - STAGGER)

    nc.finalize()
    return nc


def kernel(qkv, sim, proj_w, proj_b, logit_scale, H=None, W=None, **_):
    global LAST_RESULT
    from concourse.bass_utils import run_bass_kernel_spmd

    qkv = np.asarray(qkv, dtype=np.float32)
    sim = np.asarray(sim, dtype=np.float32)
    proj_w = np.asarray(proj_w, dtype=np.float32)
    proj_b = np.asarray(proj_b, dtype=np.float32)
    scale = float(np.exp(min(float(np.asarray(logit_scale).reshape(-1)[0]), np.log(100.0))))

    b, n, c3 = qkv.shape
    assert (b, n, c3) == (B, N, 3 * CDIM)

    # --- host: cluster sort (data-dependent reorder = the sharding step) ---
    tk = np.argmax(sim, axis=-1)                          # (b, n)
    idx = np.argsort(tk, axis=-1, kind="stable")          # (b, n)
    srt = np.take_along_axis(qkv, idx[..., None], axis=1) # (b, n, 576)
    grp = srt.reshape(NG, GS, 3 * CDIM)                   # (1024, 128, 576)

    q = grp[:, :, :CDIM].reshape(NG, GS, NUM_HEADS, HD)
    k = grp[:, :, CDIM : 2 * CDIM].reshape(NG, GS, NUM_HEADS, HD)
    # [d, g, h, s] layouts, 16KB-contiguous per (d, chunk) DMA run
    qt = np.ascontiguousarray(q.transpose(3, 0, 2, 1) * scale).astype(np.float16)
    kt = np.ascontiguousarray(k.transpose(3, 0, 2, 1)).astype(np.float16)

    # fold proj into v: vtilde[g, h, t, c] = v[g, t, h, :] @ W_h  (W_h = proj_w.T slice)
    wt = np.ascontiguousarray(proj_w.T.reshape(NUM_HEADS, HD, CDIM))
    v4 = grp[:, :, 2 * CDIM :].reshape(NG, GS, NUM_HEADS, HD)
    vt = np.empty((NG, GS, NUM_HEADS, CDIM), dtype=np.float32)
    for h in range(NUM_HEADS):
        vt[:, :, h, :] = (
            np.ascontiguousarray(v4[:, :, h, :]).reshape(NG * GS, HD) @ wt[h]
        ).reshape(NG, GS, CDIM)
    # [t, g, h, c] layout
    vt = np.ascontiguousarray(vt.transpose(1, 0, 2, 3)).astype(ml_dtypes.bfloat16)

    ident = np.eye(GS, dtype=ml_dtypes.bfloat16)

    key = "nc"
    if key not in _cache:
        _cache[key] = _build_nc()
    nc = _cache[key]

    in_maps = []
    for i in range(NCORES):
        gs_ = slice(i * GCORE, (i + 1) * GCORE)
        qs = np.ascontiguousarray(qt[:, gs_]).reshape(HD, NCHUNK, CHUNK * FW)
        ks = np.ascontiguousarray(kt[:, gs_]).reshape(HD, NCHUNK, CHUNK * FW)
        vs = np.ascontiguousarray(vt[:, gs_]).reshape(GS, NCHUNK, CHUNK * VW)
        in_maps.append({"qt": qs, "kt": ks, "vt": vs, "ident": ident})

    trace = bool(os.environ.get("BASS_TRACE"))
    res = run_bass_kernel_spmd(nc, in_maps, core_ids=list(range(NCORES)), trace=trace)
    LAST_RESULT = res

    out_sorted = np.concatenate(
        [
            np.asarray(res.results[i]["out"], dtype=np.float32).transpose(1, 0, 2)
            for i in range(NCORES)
        ],
        axis=0,
    )                                                     # (1024, 128, 192)
    out_sorted = out_sorted.reshape(B, N, CDIM) + proj_b[None, None, :]
    out = np.empty((B, N, CDIM), dtype=np.float32)
    np.put_along_axis(out, idx[..., None], out_sorted, axis=1)
    return out

